# revision 4
# baseline (speedup 1.0000x reference)
"""Transformer encoder layer (post-norm, 16 heads, d_model=1024, d_ff=4096)
on 8 Trainium2 NeuronCores.

Sharding: batch(4) x seq-half(2) -> 8 shards. Each core computes K/V for its
batch's FULL sequence (12% redundant FLOPs) and Q/attention/FFN/LN for its
1024-query half. Fully local -- no collectives.

On-chip dataflow (per core), all matmul inputs bf16, fp32 accumulation:
  XT (feature-major x.T, bf16)  --PE-->  KT (feat-major), QT (feat-major),
                                         V (token-major, +ones col per head)
  scores^T = KT_h.T @ QT_h (key-major)  --ACT exp(s/8)-->  E (bf16)
  attn/sums = E.T @ [V_h | 1]  -> normalize (DVE) -> PE-transpose -> concatT
  attn_out = concatT.T @ Wo + x_half(+bo)  -> LayerNorm1 -> norm1 (resid)
  norm1 -> PE-transpose -> norm1T -> relu(W1.T @ norm1T + b1) = relu1T
  ffn2 = relu1T.T @ W2 (+norm1 +b2) -> LayerNorm2 -> out
Softmax skips the max-subtraction (scores ~ N(0,1); exp is safe in fp32),
which is mathematically identical after normalization.
"""

import numpy as np
import ml_dtypes

B, S, D = 4, 2048, 1024
H, DK = 16, 64
DFF = 4096
SQ = S // 2          # queries per core
P = 128              # partitions
EPS = 1e-6
NCORES = 8

BF16 = ml_dtypes.bfloat16

_PROG = None  # cached compiled program


def _build_program():
    import concourse.bacc as bacc
    import concourse.tile as tile
    import concourse.mybir as mybir
    from concourse.masks import make_identity

    f32 = mybir.dt.float32
    bf16 = mybir.dt.bfloat16
    AF = mybir.ActivationFunctionType
    Alu = mybir.AluOpType

    nc = bacc.Bacc("TRN2", target_bir_lowering=False, debug=False,
                   num_devices=NCORES)

    # ---- DRAM parameters (per-core shards supplied by host) ----
    xt = nc.declare_dram_parameter("xt", [D, S], bf16, isOutput=False)        # x[b].T
    xh = nc.declare_dram_parameter("xh", [SQ, D], f32, isOutput=False)        # x_half + bo
    wq = nc.declare_dram_parameter("wq", [D, D], bf16, isOutput=False)
    wk = nc.declare_dram_parameter("wk", [D, D], bf16, isOutput=False)
    wv = nc.declare_dram_parameter("wv", [D, D], bf16, isOutput=False)
    wo = nc.declare_dram_parameter("wo", [D, D], bf16, isOutput=False)
    w1 = nc.declare_dram_parameter("w1", [D, DFF], bf16, isOutput=False)
    w2 = nc.declare_dram_parameter("w2", [DFF, D], bf16, isOutput=False)
    bq = nc.declare_dram_parameter("bq", [D], f32, isOutput=False)
    bk = nc.declare_dram_parameter("bk", [D], f32, isOutput=False)
    bvh = nc.declare_dram_parameter("bvh", [D], bf16, isOutput=False)
    b1p = nc.declare_dram_parameter("b1", [DFF], f32, isOutput=False)
    b2p = nc.declare_dram_parameter("b2", [D], f32, isOutput=False)
    a1p = nc.declare_dram_parameter("alpha1", [D], f32, isOutput=False)
    g1p = nc.declare_dram_parameter("beta1", [D], f32, isOutput=False)
    a2p = nc.declare_dram_parameter("alpha2", [D], f32, isOutput=False)
    g2p = nc.declare_dram_parameter("beta2", [D], f32, isOutput=False)
    out = nc.declare_dram_parameter("out", [SQ, D], f32, isOutput=True)

    KC = D // P          # 8 k-chunks of 128
    DCH = D // P         # 8 feature chunks
    SCH = S // P         # 16 s-chunks
    SQCH = SQ // P       # 8 query chunks
    NW = 512             # matmul free-dim tile

    import concourse.bass as bass

    def bcast(ap_1d, n):
        return bass.AP(tensor=ap_1d.tensor, offset=ap_1d.offset,
                       ap=[[0, P]] + list(ap_1d.ap[-1:]))[:, 0:n]

    with tile.TileContext(nc) as tc:
        with tc.tile_pool(name="main", bufs=1) as mp, \
             tc.tile_pool(name="stream", bufs=2) as sp, \
             tc.tile_pool(name="small", bufs=4) as smp, \
             tc.tile_pool(name="at2p", bufs=16) as at2p, \
             tc.tile_pool(name="ps", bufs=4, space="PSUM") as ps, \
             tc.tile_pool(name="psat", bufs=2, space="PSUM") as psat, \
             tc.tile_pool(name="pstr", bufs=2, space="PSUM") as pstr:

            # ---- constants ----
            ident_bf = mp.tile([P, P], bf16, tag="ident_bf")
            make_identity(nc, ident_bf)
            ident_f32 = mp.tile([P, P], f32, tag="ident_f32")
            make_identity(nc, ident_f32)

            bq_sb = mp.tile([P, DCH], f32, tag="bq")
            nc.sync.dma_start(out=bq_sb, in_=bq[:].rearrange("(c p) -> p c", p=P))
            bk_sb = mp.tile([P, DCH], f32, tag="bk")
            nc.sync.dma_start(out=bk_sb, in_=bk[:].rearrange("(c p) -> p c", p=P))
            b1_sb = mp.tile([P, DFF // P], f32, tag="b1")
            nc.sync.dma_start(out=b1_sb, in_=b1p[:].rearrange("(c p) -> p c", p=P))
            bv_b = mp.tile([P, D], bf16, tag="bv_b")
            nc.sync.dma_start(out=bv_b, in_=bcast(bvh[:], D))
            b2_b = mp.tile([P, D], f32, tag="b2_b")
            nc.sync.dma_start(out=b2_b, in_=bcast(b2p[:], D))
            a1_b = mp.tile([P, D], f32, tag="a1_b")
            nc.sync.dma_start(out=a1_b, in_=bcast(a1p[:], D))
            g1_b = mp.tile([P, D], f32, tag="g1_b")
            nc.sync.dma_start(out=g1_b, in_=bcast(g1p[:], D))
            a2_b = mp.tile([P, D], f32, tag="a1_b")
            nc.sync.dma_start(out=a2_b, in_=bcast(a2p[:], D))
            g2_b = mp.tile([P, D], f32, tag="g1_b")
            nc.sync.dma_start(out=g2_b, in_=bcast(g2p[:], D))

            # ---- load x.T (feature-major) ----
            xtb = mp.tile([P, KC, S], bf16, tag="slotA")
            nc.sync.dma_start(out=xtb, in_=xt[:, :].rearrange("(c p) s -> p c s", p=P))

            ktb = mp.tile([P, DCH, S], bf16, tag="slotB")
            qtb = mp.tile([P, DCH, SQ], bf16, tag="slotC")
            vaug = mp.tile([P, SCH, H * (DK + 1)], bf16, tag="slotD")
            # ones column per head (softmax denominator via augmented matmul)
            va_view = vaug.rearrange("p s (h w) -> p s h w", w=DK + 1)
            nc.vector.memset(va_view[:, :, :, DK:DK + 1], 1.0)

            # ================= QKV projections =================
            with nc.named_scope("qkv"):
                # QT: feature-major [D, SQ]
                wq_sb = sp.tile([P, KC, D], bf16, tag="slotE")
                nc.sync.dma_start(out=wq_sb, in_=wq[:, :].rearrange("(c p) n -> p c n", p=P))
                hoff = 0  # host supplies this core's query half in qslice of xt
                for dch in range(DCH):
                    pts = [ps.tile([P, NW], f32, tag="mm", name=f"pt{i}") for i in range(2)]
                    for kc in range(KC):
                        for n in range(2):
                            nc.tensor.matmul(
                                pts[n],
                                wq_sb[:, kc, dch * P:(dch + 1) * P],
                                xtb[:, kc, n * NW:(n + 1) * NW],
                                start=(kc == 0), stop=(kc == KC - 1))
                    for n in range(2):
                        nc.scalar.activation(
                            qtb[:, dch, n * NW:(n + 1) * NW], pts[n],
                            AF.Identity, bias=bq_sb[:, dch:dch + 1])

                # KT: feature-major [D, S]
                wk_sb = sp.tile([P, KC, D], bf16, tag="slotE")
                nc.sync.dma_start(out=wk_sb, in_=wk[:, :].rearrange("(c p) n -> p c n", p=P))
                for dch in range(DCH):
                    pts = [ps.tile([P, NW], f32, tag="mm", name=f"pt{i}") for i in range(4)]
                    for kc in range(KC):
                        for n in range(4):
                            nc.tensor.matmul(
                                pts[n],
                                wk_sb[:, kc, dch * P:(dch + 1) * P],
                                xtb[:, kc, n * NW:(n + 1) * NW],
                                start=(kc == 0), stop=(kc == KC - 1))
                    for n in range(4):
                        nc.scalar.activation(
                            ktb[:, dch, n * NW:(n + 1) * NW], pts[n],
                            AF.Identity, bias=bk_sb[:, dch:dch + 1])

                # V: token-major [S, D] scattered into vaug (+bv)
                wv_sb = sp.tile([P, KC, D], bf16, tag="slotE")
                nc.sync.dma_start(out=wv_sb, in_=wv[:, :].rearrange("(c p) n -> p c n", p=P))
                for sch in range(SCH):
                    pts = [ps.tile([P, NW], f32, tag="mm", name=f"pt{i}") for i in range(2)]
                    for kc in range(KC):
                        for n in range(2):
                            nc.tensor.matmul(
                                pts[n],
                                xtb[:, kc, sch * P:(sch + 1) * P],
                                wv_sb[:, kc, n * NW:(n + 1) * NW],
                                start=(kc == 0), stop=(kc == KC - 1))
                    for n in range(2):
                        h0 = n * (NW // DK)  # 8 heads per 512 cols
                        nc.vector.tensor_add(
                            va_view[:, sch, h0:h0 + 8, 0:DK],
                            pts[n].rearrange("p (h w) -> p h w", w=DK),
                            bv_b[:, n * NW:(n + 1) * NW].rearrange("p (h w) -> p h w", w=DK))

            # ================= attention =================
            concatT = mp.tile([P, DCH, SQ], bf16, tag="slotF")
            with nc.named_scope("attn"):
                for hp in range(H // 2):
                    at2 = {}  # (j, q) -> [P, P] bf16 pair-assembled attn
                    for j in range(2):
                        for q in range(4):
                            at2[(j, q)] = at2p.tile([P, P], bf16, tag="at2", name=f"at2_{j}_{q}")
                    for hsub in range(2):
                        h = 2 * hp + hsub
                        khc, koff = h // 2, (h % 2) * DK
                        for j in range(2):
                            et = sp.tile([P, SCH, NW], bf16, tag="slotE")
                            for sch in range(SCH):
                                pt = ps.tile([P, NW], f32, tag="mm")
                                nc.tensor.matmul(
                                    pt,
                                    ktb[koff:koff + DK, khc, sch * P:(sch + 1) * P],
                                    qtb[koff:koff + DK, khc, j * NW:(j + 1) * NW],
                                    start=True, stop=True)
                                nc.scalar.activation(
                                    et[:, sch, :], pt, AF.Exp,
                                    scale=float(1.0 / np.sqrt(DK)))
                            for q in range(4):
                                pat = psat.tile([P, DK + 1], f32, tag="at")
                                for sch in range(SCH):
                                    nc.tensor.matmul(
                                        pat,
                                        et[:, sch, q * P:(q + 1) * P],
                                        vaug[:, sch, h * (DK + 1):(h + 1) * (DK + 1)],
                                        start=(sch == 0), stop=(sch == SCH - 1))
                                rec = smp.tile([P, 1], f32, tag="rec")
                                nc.vector.reciprocal(rec, pat[:, DK:DK + 1])
                                nc.vector.tensor_scalar_mul(
                                    at2[(j, q)][:, hsub * DK:(hsub + 1) * DK],
                                    pat[:, 0:DK], rec)
                    for j in range(2):
                        for q in range(4):
                            ptr = pstr.tile([P, P], bf16, tag="tr")
                            nc.tensor.transpose(ptr, at2[(j, q)], ident_bf)
                            nc.vector.tensor_copy(
                                concatT[:, hp, j * NW + q * P: j * NW + (q + 1) * P],
                                ptr)

            # ================= O-projection + LN1 =================
            norm1 = mp.tile([P, SQCH, D], f32, tag="slotA")
            norm1T = mp.tile([P, DCH, SQ], bf16, tag="slotC")

            def layer_norm(s_slices, stats_n, mean_t, std_t, rec_t, alpha_b, gamma_b,
                           out_slices, corr):
                """s_slices: list of (ap, width). out written in-place via APs."""
                stats = smp.tile([P, stats_n, 6], f32, tag="stats")
                i = 0
                for ap_, w in s_slices:
                    nsub = w // 512
                    for ssub in range(nsub):
                        nc.vector.bn_stats(stats[:, i, :], ap_[:, ssub * 512:(ssub + 1) * 512])
                        i += 1
                assert i == stats_n
                mv = smp.tile([P, 2], f32, tag="mv")
                nc.vector.bn_aggr(mv, stats)
                # unbiased std (ddof=1), eps added to std
                nc.scalar.activation(std_t, mv[:, 1:2], AF.Sqrt, scale=float(corr))
                nc.vector.tensor_scalar_add(std_t, std_t, float(EPS))
                nc.vector.reciprocal(rec_t, std_t)
                nc.vector.tensor_copy(mean_t, mv[:, 0:1])
                for (ap_, w), (oap, alo) in zip(s_slices, out_slices):
                    nc.vector.tensor_scalar(
                        oap, ap_, mean_t, rec_t,
                        op0=Alu.subtract, op1=Alu.mult)
                    nc.vector.tensor_mul(oap, oap, alpha_b[:, alo:alo + w])
                    nc.vector.tensor_add(oap, oap, gamma_b[:, alo:alo + w])

            with nc.named_scope("o_ln1"):
                wo_sb = sp.tile([P, KC, D], bf16, tag="slotE")
                nc.sync.dma_start(out=wo_sb, in_=wo[:, :].rearrange("(c p) n -> p c n", p=P))
                for sq in range(SQCH):
                    pts = [ps.tile([P, NW], f32, tag="mm", name=f"pt{i}") for i in range(2)]
                    for kc in range(KC):
                        for n in range(2):
                            nc.tensor.matmul(
                                pts[n],
                                concatT[:, kc, sq * P:(sq + 1) * P],
                                wo_sb[:, kc, n * NW:(n + 1) * NW],
                                start=(kc == 0), stop=(kc == KC - 1))
                    xh_t = smp.tile([P, D], f32, tag="tokf32")
                    nc.sync.dma_start(out=xh_t, in_=xh[sq * P:(sq + 1) * P, :])
                    s1 = smp.tile([P, D], f32, tag="tokf32")
                    for n in range(2):
                        nc.vector.tensor_add(
                            s1[:, n * NW:(n + 1) * NW], pts[n],
                            xh_t[:, n * NW:(n + 1) * NW])
                    mean_t = smp.tile([P, 1], f32, tag="mean")
                    std_t = smp.tile([P, 1], f32, tag="std")
                    rec_t = smp.tile([P, 1], f32, tag="recs")
                    layer_norm([(s1, D)], 2, mean_t, std_t, rec_t, a1_b, g1_b,
                               [(norm1[:, sq, :], 0)], D / (D - 1))
                    # transpose norm1 -> norm1T (bf16)
                    for dch in range(DCH):
                        ptr = pstr.tile([P, P], f32, tag="tr")
                        nc.tensor.transpose(
                            ptr, norm1[:, sq, dch * P:(dch + 1) * P], ident_f32)
                        nc.vector.tensor_copy(
                            norm1T[:, dch, sq * P:(sq + 1) * P], ptr)

            # ================= FFN =================
            relu0 = mp.tile([P, 16, SQ], bf16, tag="slotB")   # dff chunks 0..15
            relu1 = mp.tile([P, 16, SQ], bf16, tag="slotD")   # dff chunks 16..31
            with nc.named_scope("ffn1"):
                for wq4 in range(4):
                    w1_sb = sp.tile([P, KC, D], bf16, tag="slotE")
                    nc.sync.dma_start(
                        out=w1_sb,
                        in_=w1[:, wq4 * D:(wq4 + 1) * D].rearrange("(c p) n -> p c n", p=P))
                    for dsub in range(DCH):
                        dff_ch = wq4 * 8 + dsub
                        tgt = relu0 if dff_ch < 16 else relu1
                        tch = dff_ch % 16
                        pts = [ps.tile([P, NW], f32, tag="mm", name=f"pt{i}") for i in range(2)]
                        for kc in range(KC):
                            for n in range(2):
                                nc.tensor.matmul(
                                    pts[n],
                                    w1_sb[:, kc, dsub * P:(dsub + 1) * P],
                                    norm1T[:, kc, n * NW:(n + 1) * NW],
                                    start=(kc == 0), stop=(kc == KC - 1))
                        for n in range(2):
                            nc.scalar.activation(
                                tgt[:, tch, n * NW:(n + 1) * NW], pts[n],
                                AF.Relu, bias=b1_sb[:, dff_ch:dff_ch + 1])

            s2a = mp.tile([P, SQCH, 512], f32, tag="slotC")   # features 0:512
            s2b = mp.tile([P, SQCH, 512], f32, tag="slotF")   # features 512:1024
            with nc.named_scope("ffn2"):
                for ncol in range(4):
                    w2_sb = sp.tile([P, DFF // P, 256], bf16, tag="slotE")
                    nc.sync.dma_start(
                        out=w2_sb,
                        in_=w2[:, ncol * 256:(ncol + 1) * 256].rearrange(
                            "(c p) n -> p c n", p=P))
                    tgt = s2a if ncol < 2 else s2b
                    tcol = (ncol % 2) * 256
                    fcol = ncol * 256
                    for sq in range(SQCH):
                        pt = ps.tile([P, 256], f32, tag="mm")
                        for kc in range(DFF // P):
                            lhs = relu0 if kc < 16 else relu1
                            nc.tensor.matmul(
                                pt,
                                lhs[:, kc % 16, sq * P:(sq + 1) * P],
                                w2_sb[:, kc, :],
                                start=(kc == 0), stop=(kc == DFF // P - 1))
                        nc.vector.tensor_add(
                            tgt[:, sq, tcol:tcol + 256], pt,
                            norm1[:, sq, fcol:fcol + 256])
                        nc.vector.tensor_add(
                            tgt[:, sq, tcol:tcol + 256],
                            tgt[:, sq, tcol:tcol + 256],
                            b2_b[:, fcol:fcol + 256])

            with nc.named_scope("ln2"):
                for sq in range(SQCH):
                    mean_t = smp.tile([P, 1], f32, tag="mean")
                    std_t = smp.tile([P, 1], f32, tag="std")
                    rec_t = smp.tile([P, 1], f32, tag="recs")
                    layer_norm(
                        [(s2a[:, sq, :], 512), (s2b[:, sq, :], 512)], 2,
                        mean_t, std_t, rec_t, a2_b, g2_b,
                        [(s2a[:, sq, :], 0), (s2b[:, sq, :], 512)], D / (D - 1))
                    nc.sync.dma_start(
                        out=out[sq * P:(sq + 1) * P, 0:512], in_=s2a[:, sq, :])
                    nc.sync.dma_start(
                        out=out[sq * P:(sq + 1) * P, 512:1024], in_=s2b[:, sq, :])

    nc.compile()
    return nc


def _get_program():
    global _PROG
    if _PROG is None:
        _PROG = _build_program()
    return _PROG


def make_in_maps(x, Wq, bq, Wk, bk, Wv, bv, Wo, bo, alpha1, bias1, alpha2,
                 bias2, W1, b1, W2, b2):
    """Build the 8 per-core input maps. Shared arrays are reused by reference."""
    def b16(a):
        return np.ascontiguousarray(a).astype(BF16)

    shared = {
        "wq": b16(Wq), "wk": b16(Wk), "wv": b16(Wv), "wo": b16(Wo),
        "w1": b16(W1), "w2": b16(W2),
        "bq": np.asarray(bq, np.float32), "bk": np.asarray(bk, np.float32),
        "bvh": b16(bv), "b1": np.asarray(b1, np.float32),
        "b2": np.asarray(b2, np.float32),
        "alpha1": np.asarray(alpha1, np.float32),
        "beta1": np.asarray(bias1, np.float32),
        "alpha2": np.asarray(alpha2, np.float32),
        "beta2": np.asarray(bias2, np.float32),
    }
    x = np.asarray(x, np.float32)
    bo = np.asarray(bo, np.float32)
    in_maps = []
    for c in range(NCORES):
        b, j = c // 2, c % 2
        # xt column order: this core's query half FIRST (cols 0:SQ), then the
        # other half -- so Q reads cols 0:SQ while K/V still see the full seq.
        xb = x[b]
        if j == 0:
            xt_np = xb.T
        else:
            xt_np = np.concatenate([xb[SQ:].T, xb[:SQ].T], axis=1)
        m = dict(shared)
        m["xt"] = b16(xt_np)
        m["xh"] = np.ascontiguousarray(xb[j * SQ:(j + 1) * SQ] + bo[None, :],
                                       dtype=np.float32)
        in_maps.append(m)
    return in_maps


def kernel(**inputs):
    from concourse.bass_utils import run_bass_kernel_spmd

    nc = _get_program()
    in_maps = make_in_maps(**inputs)
    res = run_bass_kernel_spmd(nc, in_maps, core_ids=list(range(NCORES)))
    out = np.empty((B, S, D), np.float32)
    for c in range(NCORES):
        b, j = c // 2, c % 2
        out[b, j * SQ:(j + 1) * SQ, :] = res.results[c]["out"]
    return out


# revision 7
# speedup vs baseline: 1.1931x; 1.1931x over previous
"""Transformer encoder layer (post-norm, 16 heads, d_model=1024, d_ff=4096)
on 8 Trainium2 NeuronCores.

Sharding: batch(4) x seq-half(2) -> 8 shards. Each core computes K/V for its
batch's FULL sequence (12% redundant FLOPs) and Q/attention/FFN/LN for its
1024-query half. Fully local -- no collectives.

On-chip dataflow (per core), all matmul inputs bf16, fp32 accumulation:
  XT (feature-major x.T, bf16)  --PE-->  KT (feat-major), QT (feat-major),
                                         V (token-major, +ones col per head)
  scores^T = KT_h.T @ QT_h (key-major)  --ACT exp(s/8)-->  E (bf16)
  attn/sums = E.T @ [V_h | 1]  -> normalize (DVE) -> PE-transpose -> concatT
  attn_out = concatT.T @ Wo + x_half(+bo)  -> LayerNorm1 -> norm1 (resid)
  norm1 -> PE-transpose -> norm1T -> relu(W1.T @ norm1T + b1) = relu1T
  ffn2 = relu1T.T @ W2 (+norm1 +b2) -> LayerNorm2 -> out
Softmax skips the max-subtraction (scores ~ N(0,1); exp is safe in fp32),
which is mathematically identical after normalization.
"""

import numpy as np
import ml_dtypes

B, S, D = 4, 2048, 1024
H, DK = 16, 64
DFF = 4096
SQ = S // 2          # queries per core
P = 128              # partitions
EPS = 1e-6
NCORES = 8

BF16 = ml_dtypes.bfloat16

_PROG = None  # cached compiled program


def _build_program():
    import concourse.bacc as bacc
    import concourse.tile as tile
    import concourse.mybir as mybir
    from concourse.masks import make_identity

    f32 = mybir.dt.float32
    bf16 = mybir.dt.bfloat16
    AF = mybir.ActivationFunctionType
    Alu = mybir.AluOpType

    nc = bacc.Bacc("TRN2", target_bir_lowering=False, debug=False,
                   num_devices=NCORES)

    # ---- DRAM parameters (per-core shards supplied by host) ----
    xt = nc.declare_dram_parameter("xt", [D, S], bf16, isOutput=False)        # x[b].T
    xh = nc.declare_dram_parameter("xh", [SQ, D], f32, isOutput=False)        # x_half + bo
    wq = nc.declare_dram_parameter("wq", [D, D], bf16, isOutput=False)
    wk = nc.declare_dram_parameter("wk", [D, D], bf16, isOutput=False)
    wv = nc.declare_dram_parameter("wv", [D, D], bf16, isOutput=False)
    wo = nc.declare_dram_parameter("wo", [D, D], bf16, isOutput=False)
    w1 = nc.declare_dram_parameter("w1", [D, DFF], bf16, isOutput=False)
    w2 = nc.declare_dram_parameter("w2", [DFF, D], bf16, isOutput=False)
    bq = nc.declare_dram_parameter("bq", [D], f32, isOutput=False)
    bk = nc.declare_dram_parameter("bk", [D], f32, isOutput=False)
    bvh = nc.declare_dram_parameter("bvh", [D], bf16, isOutput=False)
    b1p = nc.declare_dram_parameter("b1", [DFF], f32, isOutput=False)
    b2p = nc.declare_dram_parameter("b2", [D], f32, isOutput=False)
    a1p = nc.declare_dram_parameter("alpha1", [D], f32, isOutput=False)
    g1p = nc.declare_dram_parameter("beta1", [D], f32, isOutput=False)
    a2p = nc.declare_dram_parameter("alpha2", [D], f32, isOutput=False)
    g2p = nc.declare_dram_parameter("beta2", [D], f32, isOutput=False)
    out = nc.declare_dram_parameter("out", [SQ, D], f32, isOutput=True)

    KC = D // P          # 8 k-chunks of 128
    DCH = D // P         # 8 feature chunks
    SCH = S // P         # 16 s-chunks
    SQCH = SQ // P       # 8 query chunks
    NW = 512             # matmul free-dim tile

    import concourse.bass as bass

    def bcast(ap_1d, n):
        return bass.AP(tensor=ap_1d.tensor, offset=ap_1d.offset,
                       ap=[[0, P]] + list(ap_1d.ap[-1:]))[:, 0:n]

    with tile.TileContext(nc) as tc:
        with tc.tile_pool(name="main", bufs=1) as mp, \
             tc.tile_pool(name="stream", bufs=2) as sp, \
             tc.tile_pool(name="small", bufs=4) as smp, \
             tc.tile_pool(name="at2p", bufs=14) as at2p, \
             tc.tile_pool(name="tokp", bufs=3) as tokp, \
             tc.tile_pool(name="ps", bufs=4, space="PSUM") as ps, \
             tc.tile_pool(name="psat", bufs=2, space="PSUM") as psat, \
             tc.tile_pool(name="pstr", bufs=2, space="PSUM") as pstr:

            # ---- constants ----
            ident_bf = mp.tile([P, P], bf16, tag="ident_bf")
            make_identity(nc, ident_bf)
            ident_f32 = mp.tile([P, P], f32, tag="ident_f32")
            make_identity(nc, ident_f32)

            bq_sb = mp.tile([P, DCH], f32, tag="bq")
            nc.sync.dma_start(out=bq_sb, in_=bq[:].rearrange("(c p) -> p c", p=P))
            bk_sb = mp.tile([P, DCH], f32, tag="bk")
            nc.sync.dma_start(out=bk_sb, in_=bk[:].rearrange("(c p) -> p c", p=P))
            b1_sb = mp.tile([P, DFF // P], f32, tag="b1")
            nc.sync.dma_start(out=b1_sb, in_=b1p[:].rearrange("(c p) -> p c", p=P))
            bv_b = mp.tile([P, D], bf16, tag="bv_b")
            nc.sync.dma_start(out=bv_b, in_=bcast(bvh[:], D))
            b2_b = mp.tile([P, D], f32, tag="b2_b")
            nc.sync.dma_start(out=b2_b, in_=bcast(b2p[:], D))
            a1_b = mp.tile([P, D], f32, tag="a1_b")
            nc.sync.dma_start(out=a1_b, in_=bcast(a1p[:], D))
            g1_b = mp.tile([P, D], f32, tag="g1_b")
            nc.sync.dma_start(out=g1_b, in_=bcast(g1p[:], D))
            a2_b = mp.tile([P, D], f32, tag="a1_b")
            nc.sync.dma_start(out=a2_b, in_=bcast(a2p[:], D))
            g2_b = mp.tile([P, D], f32, tag="g1_b")
            nc.sync.dma_start(out=g2_b, in_=bcast(g2p[:], D))

            # ---- load x.T (feature-major) ----
            xtb = mp.tile([P, KC, S], bf16, tag="slotA")
            nc.sync.dma_start(out=xtb, in_=xt[:, :].rearrange("(c p) s -> p c s", p=P))

            ktb = mp.tile([P, DCH, S], bf16, tag="slotB")
            qtb = mp.tile([P, H, SQ], bf16, tag="slotC")
            nc.vector.memset(qtb, 0.0)
            vaug = mp.tile([P, SCH, H * (DK + 1)], bf16, tag="slotD")
            # ones column per head (softmax denominator via augmented matmul)
            va_view = vaug.rearrange("p s (h w) -> p s h w", w=DK + 1)
            nc.vector.memset(va_view[:, :, :, DK:DK + 1], 1.0)

            # ================= QKV projections =================
            with nc.named_scope("qkv"):
                # QT: feature-major [D, SQ]
                wq_sb = sp.tile([P, KC, D], bf16, tag="slotE")
                nc.sync.dma_start(out=wq_sb, in_=wq[:, :].rearrange("(c p) n -> p c n", p=P))
                hoff = 0  # host supplies this core's query half in qslice of xt
                for dch in range(DCH):
                    pts = [ps.tile([P, NW], f32, tag="mm", name=f"pt{i}") for i in range(2)]
                    for kc in range(KC):
                        for n in range(2):
                            nc.tensor.matmul(
                                pts[n],
                                wq_sb[:, kc, dch * P:(dch + 1) * P],
                                xtb[:, kc, n * NW:(n + 1) * NW],
                                start=(kc == 0), stop=(kc == KC - 1))
                    for n in range(2):
                        nc.scalar.activation(
                            qtb[0:64, 2 * dch, n * NW:(n + 1) * NW],
                            pts[n][0:64, :],
                            AF.Identity, bias=bq_sb[0:64, dch:dch + 1])
                        nc.scalar.activation(
                            qtb[64:128, 2 * dch + 1, n * NW:(n + 1) * NW],
                            pts[n][64:128, :],
                            AF.Identity, bias=bq_sb[64:128, dch:dch + 1])

                # KT: feature-major [D, S]
                wk_sb = sp.tile([P, KC, D], bf16, tag="slotE")
                nc.sync.dma_start(out=wk_sb, in_=wk[:, :].rearrange("(c p) n -> p c n", p=P))
                for dch in range(DCH):
                    pts = [ps.tile([P, NW], f32, tag="mm", name=f"pt{i}") for i in range(4)]
                    for kc in range(KC):
                        for n in range(4):
                            nc.tensor.matmul(
                                pts[n],
                                wk_sb[:, kc, dch * P:(dch + 1) * P],
                                xtb[:, kc, n * NW:(n + 1) * NW],
                                start=(kc == 0), stop=(kc == KC - 1))
                    for n in range(4):
                        nc.scalar.activation(
                            ktb[:, dch, n * NW:(n + 1) * NW], pts[n],
                            AF.Identity, bias=bk_sb[:, dch:dch + 1])

                # V: token-major [S, D] scattered into vaug (+bv)
                wv_sb = sp.tile([P, KC, D], bf16, tag="slotE")
                nc.sync.dma_start(out=wv_sb, in_=wv[:, :].rearrange("(c p) n -> p c n", p=P))
                for sch in range(SCH):
                    pts = [ps.tile([P, NW], f32, tag="mm", name=f"pt{i}") for i in range(2)]
                    for kc in range(KC):
                        for n in range(2):
                            nc.tensor.matmul(
                                pts[n],
                                xtb[:, kc, sch * P:(sch + 1) * P],
                                wv_sb[:, kc, n * NW:(n + 1) * NW],
                                start=(kc == 0), stop=(kc == KC - 1))
                    for n in range(2):
                        h0 = n * (NW // DK)  # 8 heads per 512 cols
                        nc.vector.tensor_add(
                            va_view[:, sch, h0:h0 + 8, 0:DK],
                            pts[n].rearrange("p (h w) -> p h w", w=DK),
                            bv_b[:, n * NW:(n + 1) * NW].rearrange("p (h w) -> p h w", w=DK))

            # ================= attention =================
            concatT = mp.tile([P, DCH, SQ], bf16, tag="slotF")
            with nc.named_scope("attn"):
                for hp in range(H // 2):
                    at2 = {}  # (j, q) -> [P, P] bf16 pair-assembled attn
                    for j in range(2):
                        for q in range(4):
                            at2[(j, q)] = at2p.tile([P, P], bf16, tag="at2", name=f"at2_{j}_{q}")
                    for hsub in range(2):
                        h = 2 * hp + hsub
                        khc, koff = h // 2, (h % 2) * DK
                        for j in range(2):
                            et = sp.tile([P, SCH, NW], bf16, tag="slotE")
                            for sch in range(SCH):
                                pt = ps.tile([P, NW], f32, tag="mm")
                                nc.tensor.matmul(
                                    pt,
                                    ktb[:, khc, sch * P:(sch + 1) * P],
                                    qtb[:, h, j * NW:(j + 1) * NW],
                                    start=True, stop=True)
                                nc.scalar.activation(
                                    et[:, sch, :], pt, AF.Exp,
                                    scale=float(1.0 / np.sqrt(DK)))
                            for q in range(4):
                                pat = psat.tile([P, DK + 1], f32, tag="at")
                                for sch in range(SCH):
                                    nc.tensor.matmul(
                                        pat,
                                        et[:, sch, q * P:(q + 1) * P],
                                        vaug[:, sch, h * (DK + 1):(h + 1) * (DK + 1)],
                                        start=(sch == 0), stop=(sch == SCH - 1))
                                rec = smp.tile([P, 1], f32, tag="rec")
                                nc.vector.reciprocal(rec, pat[:, DK:DK + 1])
                                nc.vector.tensor_scalar_mul(
                                    at2[(j, q)][:, hsub * DK:(hsub + 1) * DK],
                                    pat[:, 0:DK], rec)
                    for j in range(2):
                        for q in range(4):
                            ptr = pstr.tile([P, P], bf16, tag="tr")
                            nc.tensor.transpose(ptr, at2[(j, q)], ident_bf)
                            nc.vector.tensor_copy(
                                concatT[:, hp, j * NW + q * P: j * NW + (q + 1) * P],
                                ptr)

            # ================= O-projection + LN1 =================
            norm1 = mp.tile([P, SQCH, D], f32, tag="slotA")
            norm1T = mp.tile([P, DCH, SQ], bf16, tag="slotC")

            def layer_norm(s_slices, stats_n, mean_t, std_t, rec_t, alpha_b, gamma_b,
                           out_slices, corr):
                """s_slices: list of (ap, width). out written in-place via APs."""
                stats = smp.tile([P, stats_n, 6], f32, tag="stats")
                i = 0
                for ap_, w in s_slices:
                    nsub = w // 512
                    for ssub in range(nsub):
                        nc.vector.bn_stats(stats[:, i, :], ap_[:, ssub * 512:(ssub + 1) * 512])
                        i += 1
                assert i == stats_n
                mv = smp.tile([P, 2], f32, tag="mv")
                nc.vector.bn_aggr(mv, stats)
                # unbiased std (ddof=1), eps added to std
                nc.scalar.activation(std_t, mv[:, 1:2], AF.Sqrt, scale=float(corr))
                nc.vector.tensor_scalar_add(std_t, std_t, float(EPS))
                nc.vector.reciprocal(rec_t, std_t)
                nc.vector.tensor_copy(mean_t, mv[:, 0:1])
                for (ap_, w), (oap, alo) in zip(s_slices, out_slices):
                    nc.vector.tensor_scalar(
                        oap, ap_, mean_t, rec_t,
                        op0=Alu.subtract, op1=Alu.mult)
                    nc.vector.tensor_mul(oap, oap, alpha_b[:, alo:alo + w])
                    nc.vector.tensor_add(oap, oap, gamma_b[:, alo:alo + w])

            with nc.named_scope("o_ln1"):
                wo_sb = sp.tile([P, KC, D], bf16, tag="slotE")
                nc.sync.dma_start(out=wo_sb, in_=wo[:, :].rearrange("(c p) n -> p c n", p=P))
                for sq in range(SQCH):
                    pts = [ps.tile([P, NW], f32, tag="mm", name=f"pt{i}") for i in range(2)]
                    for kc in range(KC):
                        for n in range(2):
                            nc.tensor.matmul(
                                pts[n],
                                concatT[:, kc, sq * P:(sq + 1) * P],
                                wo_sb[:, kc, n * NW:(n + 1) * NW],
                                start=(kc == 0), stop=(kc == KC - 1))
                    xh_t = tokp.tile([P, D], f32, tag="tokf32")
                    nc.sync.dma_start(out=xh_t, in_=xh[sq * P:(sq + 1) * P, :])
                    s1 = tokp.tile([P, D], f32, tag="tokf32")
                    for n in range(2):
                        nc.vector.tensor_add(
                            s1[:, n * NW:(n + 1) * NW], pts[n],
                            xh_t[:, n * NW:(n + 1) * NW])
                    mean_t = smp.tile([P, 1], f32, tag="mean")
                    std_t = smp.tile([P, 1], f32, tag="std")
                    rec_t = smp.tile([P, 1], f32, tag="recs")
                    layer_norm([(s1, D)], 2, mean_t, std_t, rec_t, a1_b, g1_b,
                               [(norm1[:, sq, :], 0)], D / (D - 1))
                    # transpose norm1 -> norm1T (bf16)
                    for dch in range(DCH):
                        ptr = pstr.tile([P, P], f32, tag="tr")
                        nc.tensor.transpose(
                            ptr, norm1[:, sq, dch * P:(dch + 1) * P], ident_f32)
                        nc.vector.tensor_copy(
                            norm1T[:, dch, sq * P:(sq + 1) * P], ptr)

            # ================= FFN =================
            relu0 = mp.tile([P, 16, SQ], bf16, tag="slotB")   # dff chunks 0..15
            relu1 = mp.tile([P, 16, SQ], bf16, tag="slotD")   # dff chunks 16..31
            with nc.named_scope("ffn1"):
                for wq4 in range(4):
                    w1_sb = sp.tile([P, KC, D], bf16, tag="slotE")
                    nc.sync.dma_start(
                        out=w1_sb,
                        in_=w1[:, wq4 * D:(wq4 + 1) * D].rearrange("(c p) n -> p c n", p=P))
                    for dsub in range(DCH):
                        dff_ch = wq4 * 8 + dsub
                        tgt = relu0 if dff_ch < 16 else relu1
                        tch = dff_ch % 16
                        pts = [ps.tile([P, NW], f32, tag="mm", name=f"pt{i}") for i in range(2)]
                        for kc in range(KC):
                            for n in range(2):
                                nc.tensor.matmul(
                                    pts[n],
                                    w1_sb[:, kc, dsub * P:(dsub + 1) * P],
                                    norm1T[:, kc, n * NW:(n + 1) * NW],
                                    start=(kc == 0), stop=(kc == KC - 1))
                        for n in range(2):
                            nc.scalar.activation(
                                tgt[:, tch, n * NW:(n + 1) * NW], pts[n],
                                AF.Relu, bias=b1_sb[:, dff_ch:dff_ch + 1])

            s2a = mp.tile([P, SQCH, 512], f32, tag="slotC")   # features 0:512
            s2b = mp.tile([P, SQCH, 512], f32, tag="slotF")   # features 512:1024
            with nc.named_scope("ffn2"):
                for ncol in range(4):
                    w2_sb = sp.tile([P, DFF // P, 256], bf16, tag="slotE")
                    nc.sync.dma_start(
                        out=w2_sb,
                        in_=w2[:, ncol * 256:(ncol + 1) * 256].rearrange(
                            "(c p) n -> p c n", p=P))
                    tgt = s2a if ncol < 2 else s2b
                    tcol = (ncol % 2) * 256
                    fcol = ncol * 256
                    for sq in range(SQCH):
                        pt = ps.tile([P, 256], f32, tag="mm")
                        for kc in range(DFF // P):
                            lhs = relu0 if kc < 16 else relu1
                            nc.tensor.matmul(
                                pt,
                                lhs[:, kc % 16, sq * P:(sq + 1) * P],
                                w2_sb[:, kc, :],
                                start=(kc == 0), stop=(kc == DFF // P - 1))
                        nc.vector.tensor_add(
                            tgt[:, sq, tcol:tcol + 256], pt,
                            norm1[:, sq, fcol:fcol + 256])
                        nc.vector.tensor_add(
                            tgt[:, sq, tcol:tcol + 256],
                            tgt[:, sq, tcol:tcol + 256],
                            b2_b[:, fcol:fcol + 256])

            with nc.named_scope("ln2"):
                for sq in range(SQCH):
                    mean_t = smp.tile([P, 1], f32, tag="mean")
                    std_t = smp.tile([P, 1], f32, tag="std")
                    rec_t = smp.tile([P, 1], f32, tag="recs")
                    layer_norm(
                        [(s2a[:, sq, :], 512), (s2b[:, sq, :], 512)], 2,
                        mean_t, std_t, rec_t, a2_b, g2_b,
                        [(s2a[:, sq, :], 0), (s2b[:, sq, :], 512)], D / (D - 1))
                    nc.sync.dma_start(
                        out=out[sq * P:(sq + 1) * P, 0:512], in_=s2a[:, sq, :])
                    nc.sync.dma_start(
                        out=out[sq * P:(sq + 1) * P, 512:1024], in_=s2b[:, sq, :])

    nc.compile()
    return nc


def _get_program():
    global _PROG
    if _PROG is None:
        _PROG = _build_program()
    return _PROG


def make_in_maps(x, Wq, bq, Wk, bk, Wv, bv, Wo, bo, alpha1, bias1, alpha2,
                 bias2, W1, b1, W2, b2):
    """Build the 8 per-core input maps. Shared arrays are reused by reference."""
    def b16(a):
        return np.ascontiguousarray(a).astype(BF16)

    shared = {
        "wq": b16(Wq), "wk": b16(Wk), "wv": b16(Wv), "wo": b16(Wo),
        "w1": b16(W1), "w2": b16(W2),
        "bq": np.asarray(bq, np.float32), "bk": np.asarray(bk, np.float32),
        "bvh": b16(bv), "b1": np.asarray(b1, np.float32),
        "b2": np.asarray(b2, np.float32),
        "alpha1": np.asarray(alpha1, np.float32),
        "beta1": np.asarray(bias1, np.float32),
        "alpha2": np.asarray(alpha2, np.float32),
        "beta2": np.asarray(bias2, np.float32),
    }
    x = np.asarray(x, np.float32)
    bo = np.asarray(bo, np.float32)
    in_maps = []
    for c in range(NCORES):
        b, j = c // 2, c % 2
        # xt column order: this core's query half FIRST (cols 0:SQ), then the
        # other half -- so Q reads cols 0:SQ while K/V still see the full seq.
        xb = x[b]
        if j == 0:
            xt_np = xb.T
        else:
            xt_np = np.concatenate([xb[SQ:].T, xb[:SQ].T], axis=1)
        m = dict(shared)
        m["xt"] = b16(xt_np)
        m["xh"] = np.ascontiguousarray(xb[j * SQ:(j + 1) * SQ] + bo[None, :],
                                       dtype=np.float32)
        in_maps.append(m)
    return in_maps


def kernel(**inputs):
    from concourse.bass_utils import run_bass_kernel_spmd

    nc = _get_program()
    in_maps = make_in_maps(**inputs)
    res = run_bass_kernel_spmd(nc, in_maps, core_ids=list(range(NCORES)))
    out = np.empty((B, S, D), np.float32)
    for c in range(NCORES):
        b, j = c // 2, c % 2
        out[b, j * SQ:(j + 1) * SQ, :] = res.results[c]["out"]
    return out
